# revision 15
# baseline (speedup 1.0000x reference)
"""Trainium2 Bass kernel for C = triu(triu(A) @ triu(B)), N=4096, fp32.

Math: with host-side triu masking of A and B, the product is upper-triangular
automatically; for output element (r, c) only k in [r, c] contributes.

Sharding (8 cores, SPMD, one NEFF): a 4x2 grid.
  - Rows: 4 row-groups, cyclic mod 4 at 128-row tile granularity. Core with
    row-group r owns row-tiles {4t + r : t = 0..7} (8 slots of 128 rows).
  - Columns: 2 column-groups by n-tile parity (h = 0 even, h = 1 odd
    128-column tiles). A core owns 16 n-tiles {2u + h}, grouped into 4
    "virtual supers" v = 0..3 of 4 owned tiles {8v + 2j + h : j = 0..3}
    (512 packed columns each).
Interleaving parities keeps the SPMD loop bounds nearly tight for both
column-groups: vsuper v needs k-tiles k <= 8v + 6 + h, the program runs the
union k <= 8v+7. Where a core's data has no work the packed operands are
zero, so the extra matmuls accumulate zeros and stay correct.

Per-core traffic (bf16): B ragged-trimmed 8.9 MB, A triu-trimmed 4.7 MB,
C out 2.4 MB (bf16, host converts back to fp32) ~= 16 MB, balancing the
~888 128^3-tile-matmul units of tensor work per core.

Schedule notes:
  - ~20 warmup matmuls on memset-zero SBUF run during the NEFF preamble /
    first-DMA window so the PE HAM clock-gate reaches 8/8 before real work.
  - vsupers processed in order 1,2,3,0: best early compute-per-DMA-byte and
    the 8-copy drain of v3 overlaps v0 instead of being a tail.
  - PSUM->SBUF copies alternate VectorE / ScalarE (ACT); the A-shard chunk
    loads are emitted between copies on the Scalar queue, so they dispatch
    just-in-time instead of competing with early B streaming.
  - C stores are batched per vsuper halves (2 GpSimd DMAs per vsuper).
"""

import sys

for _p in ("/opt/trn_rl_repo", "/root/.axon_site/_ro/trn_rl_repo"):
    if _p not in sys.path:
        sys.path.insert(0, _p)

import numpy as np

N = 4096
P = 128
NCORES = 8
NSLOT = 8  # row-tiles per core (cyclic mod 4)
NV = 4  # virtual supers per core
SW = 512  # packed columns per vsuper
KT = N // P  # 32 k-tiles
N_WARM = 10  # warmup matmuls (512 wide) to flip the HAM clock gate early
VORDER = [1, 2, 3, 0]
# B k-chunk boundaries per vsuper (~1MB DMAs; v1's split finer for startup)
BCHUNKS = {1: [0, 4, 8, 16], 2: [0, 8, 16, 24], 3: [0, 8, 16, 24, 32], 0: [0, 8]}
BMAXK = 8

_cache = {}


def _kmax(v):
    return 8 * v + 7


def _j0(k, v):
    # first owned-n-tile index j (0..3) inside vsuper v that can still have
    # a nonzero triu(B) entry at k-tile k (taking the wider h=1 parity)
    return max(0, (k - 8 * v) // 2)


def _wtiles(k, v):
    return 4 - _j0(k, v)


# --- A pack layout: k-major, slots t <= k//4, trimmed to k >= 4t ---
def _aoff(k):
    # column offset (in elements) of k-tile k's slot block in the A pack
    return 128 * sum(kk // 4 + 1 for kk in range(k))


A_COLS = _aoff(KT)  # 144 * 128 = 18432


# --- B pack layout: per vsuper, per k, ragged width (4 - j0) * 128 ---
def _boff(k, v):
    return 128 * sum(_wtiles(kk, v) for kk in range(k))


_BBASE = []
_b = 0
for _v in range(NV):
    _BBASE.append(_b)
    _b += _boff(_kmax(_v) + 1, _v)
B_COLS = _b  # 272 * 128 = 34816


# --- C pack layout: blocks (v, t) for t <= 2v+1, width (4 - j0(4t, v)) * 128
def _cwidth(v, t):
    return 128 * (4 - _j0(4 * t, v))


_CBASE = {}
_c = 0
for _v in range(NV):
    for _t in range(2 * _v + 2):
        _CBASE[(_v, _t)] = _c
        _c += _cwidth(_v, _t)
C_COLS = _c  # 9216
OT_COLS = max(
    _CBASE[(_v, 2 * _v + 1)] + _cwidth(_v, 2 * _v + 1) - _CBASE[(_v, 0)]
    for _v in range(NV)
)  # 3840


def _build():
    import concourse.bacc as bacc
    import concourse.mybir as mybir
    import concourse.tile as tile

    D = mybir.dt.bfloat16
    Copy = mybir.ActivationFunctionType.Copy

    nc = bacc.Bacc(None, target_bir_lowering=False)
    AT = nc.dram_tensor("AT", [P, A_COLS], D, kind="ExternalInput")
    Bm = nc.dram_tensor("B", [P, B_COLS], D, kind="ExternalInput")
    Cm = nc.dram_tensor("C", [P, C_COLS], D, kind="ExternalOutput")

    with tile.TileContext(nc) as tc:
        with (
            tc.tile_pool(name="w", bufs=1) as wpool,
            tc.tile_pool(name="a", bufs=4) as apool,
            tc.tile_pool(name="b", bufs=5) as bpool,
            tc.tile_pool(name="o", bufs=2) as opool,
            tc.tile_pool(name="ps", bufs=8, space="PSUM") as pspool,
        ):
            # --- PE warmup: flip HAM to 8/8 during preamble + first DMAs ---
            warm = wpool.tile([P, SW], D, tag="wm", name="warm")
            nc.vector.memset(warm[:], 0)
            wps = pspool.tile([P, SW], mybir.dt.float32, tag="ps", name="ps")
            for _ in range(N_WARM):
                nc.tensor.matmul(
                    wps[:], warm[:, :P], warm[:], start=True, stop=True
                )

            # --- A shard: 4 chunks by k-group; g0/g1 up front, g2/g3 JIT ---
            a_tiles = [None] * 4
            a_starts = [_aoff(8 * g) for g in range(5)]

            def load_a(g):
                ag = apool.tile(
                    [P, a_starts[g + 1] - a_starts[g]], D, tag=f"a{g}", name="ag"
                )
                nc.scalar.dma_start(ag[:], AT[:, a_starts[g] : a_starts[g + 1]])
                a_tiles[g] = ag

            load_a(0)
            # hold the later A chunks out of the congested early window
            # (g1/g2/g3 data is first used at ~13us / ~28us / ~42us)
            with tc.tile_wait_until(0.007):
                load_a(1)
            with tc.tile_wait_until(0.010):
                load_a(2)
            with tc.tile_wait_until(0.020):
                load_a(3)

            def a_sl(k, t):
                g = k // 8
                c0 = _aoff(k) - a_starts[g] + 128 * t
                return a_tiles[g][:, c0 : c0 + 128]

            for vi, v in enumerate(VORDER):
                kmax = _kmax(v)
                nslots = 2 * v + 2
                psums = [
                    pspool.tile([P, SW], mybir.dt.float32, tag="ps", name="ps")
                    for _ in range(nslots)
                ]
                bb = BCHUNKS[v]
                for ci, (kc, kend) in enumerate(zip(bb, bb[1:])):
                    cnt = kend - kc
                    c0 = _BBASE[v] + _boff(kc, v)
                    c1 = _BBASE[v] + _boff(kc + cnt, v)
                    bt = bpool.tile([P, BMAXK * SW], D, tag="b", name="bt")
                    beng = nc.sync if (vi + ci) % 2 == 0 else nc.gpsimd
                    beng.dma_start(bt[:, : c1 - c0], Bm[:, c0:c1])
                    for k in range(kc, kc + cnt):
                        w0 = 128 * _j0(k, v)
                        b0 = _boff(k, v) - _boff(kc, v)
                        bw = 128 * _wtiles(k, v)
                        for t in range(k // 4 + 1):
                            nc.tensor.matmul(
                                psums[t][:, w0:SW],
                                a_sl(k, t),
                                bt[:, b0 : b0 + bw],
                                start=(k == 4 * t),
                                stop=(k == kmax),
                            )
                # drain PSUM -> SBUF (bf16), alternating Vector/Scalar, then
                # store in two batched DMAs on the GpSimd queue
                ot = opool.tile([P, OT_COLS], D, tag="o", name="ot")
                base = _CBASE[(v, 0)]
                for t in range(nslots):
                    w0 = 128 * _j0(4 * t, v)
                    cw = _cwidth(v, t)
                    l0 = _CBASE[(v, t)] - base
                    if t % 2 == 0:
                        nc.vector.tensor_copy(
                            ot[:, l0 : l0 + cw], psums[t][:, w0:SW]
                        )
                    else:
                        nc.scalar.activation(
                            ot[:, l0 : l0 + cw], psums[t][:, w0:SW], Copy
                        )
                half = nslots // 2
                lmid = _CBASE[(v, half)] - base
                lend = _CBASE[(v, nslots - 1)] - base + _cwidth(v, nslots - 1)
                # early vsupers' stores wait out the congested mid-kernel
                # window (B + late-A streaming); later ones go immediately
                cwait = {0: 0.042, 1: 0.050}.get(vi)
                ceng = nc.scalar
                with tc.tile_wait_until(cwait or 0, enable=cwait is not None):
                    ceng.dma_start(Cm[:, base : base + lmid], ot[:, :lmid])
                    ceng.dma_start(
                        Cm[:, base + lmid : base + lend], ot[:, lmid:lend]
                    )
    nc.compile()
    return nc


def _get_nc():
    if "nc" not in _cache:
        _cache["nc"] = _build()
    return _cache["nc"]


def _make_in_maps(A, B):
    import ml_dtypes

    bf16 = np.dtype(ml_dtypes.bfloat16)
    A = np.asarray(A, dtype=np.float32)
    B = np.asarray(B, dtype=np.float32)
    Au = np.triu(A).astype(bf16)
    Bu = np.triu(B).astype(bf16)

    # A packs per row-group r: [p, k-major slots]
    a_packs = []
    for r in range(4):
        ATr = np.zeros((P, A_COLS), dtype=bf16)
        for k in range(KT):
            base = _aoff(k)
            for t in range(k // 4 + 1):
                m = 4 * t + r
                # lhsT[p, ml] = Au[128*m + ml, 128*k + p]
                ATr[:, base + 128 * t : base + 128 * (t + 1)] = Au[
                    128 * m : 128 * m + 128, 128 * k : 128 * k + 128
                ].T
        a_packs.append(ATr)

    # B packs per column parity h
    b_packs = []
    for h in range(2):
        Bh = np.zeros((P, B_COLS), dtype=bf16)
        for v in range(NV):
            for k in range(_kmax(v) + 1):
                base = _BBASE[v] + _boff(k, v)
                for i, j in enumerate(range(_j0(k, v), 4)):
                    n = 8 * v + 2 * j + h
                    Bh[:, base + 128 * i : base + 128 * (i + 1)] = Bu[
                        128 * k : 128 * k + 128, 128 * n : 128 * n + 128
                    ]
        b_packs.append(Bh)

    in_maps = []
    for j in range(NCORES):
        r, h = j % 4, j // 4
        in_maps.append({"AT": a_packs[r], "B": b_packs[h]})
    return in_maps


def kernel(A, B):
    from concourse.bass_utils import run_bass_kernel_spmd

    in_maps = _make_in_maps(A, B)
    nc = _get_nc()
    res = run_bass_kernel_spmd(nc, in_maps, core_ids=list(range(NCORES)))

    C = np.zeros((N, N), dtype=np.float32)
    for jcore in range(NCORES):
        r, h = jcore % 4, jcore // 4
        Cj = np.asarray(res.results[jcore]["C"]).astype(np.float32)
        for v in range(NV):
            for t in range(2 * v + 2):
                m = 4 * t + r
                cb = _CBASE[(v, t)]
                for i, j in enumerate(range(_j0(4 * t, v), 4)):
                    n = 8 * v + 2 * j + h
                    C[128 * m : 128 * m + 128, 128 * n : 128 * n + 128] = Cj[
                        :, cb + 128 * i : cb + 128 * (i + 1)
                    ]
    return C


# revision 16
# speedup vs baseline: 1.1309x; 1.1309x over previous
"""Trainium2 Bass kernel for C = triu(triu(A) @ triu(B)), N=4096, fp32.

Math: with host-side triu masking of A and B, the product is upper-triangular
automatically; for output element (r, c) only k in [r, c] contributes.

Sharding (8 cores, SPMD, one NEFF): a 4x2 grid.
  - Rows: 4 row-groups, cyclic mod 4 at 128-row tile granularity. Core with
    row-group r owns row-tiles {4t + r : t = 0..7} (8 slots of 128 rows).
  - Columns: 2 column-groups by n-tile parity (h = 0 even, h = 1 odd
    128-column tiles). A core owns 16 n-tiles {2u + h}, grouped into 4
    "virtual supers" v = 0..3 of 4 owned tiles {8v + 2j + h : j = 0..3}
    (512 packed columns each).
Interleaving parities keeps the SPMD loop bounds nearly tight for both
column-groups: vsuper v needs k-tiles k <= 8v + 6 + h, the program runs the
union k <= 8v+7. Where a core's data has no work the packed operands are
zero, so the extra matmuls accumulate zeros and stay correct.

Per-core traffic (bf16): B ragged-trimmed 8.9 MB, A triu-trimmed 4.7 MB,
C out 2.4 MB (bf16, host converts back to fp32) ~= 16 MB, balancing the
~888 128^3-tile-matmul units of tensor work per core.

Schedule notes:
  - ~20 warmup matmuls on memset-zero SBUF run during the NEFF preamble /
    first-DMA window so the PE HAM clock-gate reaches 8/8 before real work.
  - vsupers processed in order 1,2,3,0: best early compute-per-DMA-byte and
    the 8-copy drain of v3 overlaps v0 instead of being a tail.
  - PSUM->SBUF copies alternate VectorE / ScalarE (ACT); the A-shard chunk
    loads are emitted between copies on the Scalar queue, so they dispatch
    just-in-time instead of competing with early B streaming.
  - C stores are batched per vsuper halves (2 GpSimd DMAs per vsuper).
"""

import sys

for _p in ("/opt/trn_rl_repo", "/root/.axon_site/_ro/trn_rl_repo"):
    if _p not in sys.path:
        sys.path.insert(0, _p)

import numpy as np

N = 4096
P = 128
NCORES = 8
NSLOT = 8  # row-tiles per core (cyclic mod 4)
NV = 4  # virtual supers per core
SW = 512  # packed columns per vsuper
KT = N // P  # 32 k-tiles
N_WARM = 10  # warmup matmuls (512 wide) to flip the HAM clock gate early
VORDER = [1, 2, 3, 0]
# B k-chunk boundaries per vsuper (~1MB DMAs; v1's split finer for startup)
BCHUNKS = {1: [0, 4, 8, 16], 2: [0, 8, 16, 24], 3: [0, 8, 16, 24, 32], 0: [0, 8]}
BMAXK = 8

_cache = {}


def _kmax(v):
    return 8 * v + 7


def _j0(k, v):
    # first owned-n-tile index j (0..3) inside vsuper v that can still have
    # a nonzero triu(B) entry at k-tile k (taking the wider h=1 parity)
    return max(0, (k - 8 * v) // 2)


def _wtiles(k, v):
    return 4 - _j0(k, v)


# --- A pack layout: k-major, slots t <= k//4, trimmed to k >= 4t ---
def _aoff(k):
    # column offset (in elements) of k-tile k's slot block in the A pack
    return 128 * sum(kk // 4 + 1 for kk in range(k))


A_COLS = _aoff(KT)  # 144 * 128 = 18432


# --- B pack layout: per vsuper, per k, ragged width (4 - j0) * 128 ---
def _boff(k, v):
    return 128 * sum(_wtiles(kk, v) for kk in range(k))


_BBASE = []
_b = 0
for _v in range(NV):
    _BBASE.append(_b)
    _b += _boff(_kmax(_v) + 1, _v)
B_COLS = _b  # 272 * 128 = 34816


# --- C pack layout: blocks (v, t) for t <= 2v+1, width (4 - j0(4t, v)) * 128
def _cwidth(v, t):
    return 128 * (4 - _j0(4 * t, v))


_CBASE = {}
_c = 0
for _v in range(NV):
    for _t in range(2 * _v + 2):
        _CBASE[(_v, _t)] = _c
        _c += _cwidth(_v, _t)
C_COLS = _c  # 9216
OT_COLS = max(
    _CBASE[(_v, 2 * _v + 1)] + _cwidth(_v, 2 * _v + 1) - _CBASE[(_v, 0)]
    for _v in range(NV)
)  # 3840


def _build():
    import concourse.bacc as bacc
    import concourse.mybir as mybir
    import concourse.tile as tile

    D = mybir.dt.bfloat16
    Copy = mybir.ActivationFunctionType.Copy

    nc = bacc.Bacc(None, target_bir_lowering=False)
    AT = nc.dram_tensor("AT", [P, A_COLS], D, kind="ExternalInput")
    Bm = nc.dram_tensor("B", [P, B_COLS], D, kind="ExternalInput")
    Cm = nc.dram_tensor("C", [P, C_COLS], D, kind="ExternalOutput")

    with tile.TileContext(nc) as tc:
        with (
            tc.tile_pool(name="w", bufs=1) as wpool,
            tc.tile_pool(name="a", bufs=4) as apool,
            tc.tile_pool(name="b", bufs=5) as bpool,
            tc.tile_pool(name="o", bufs=2) as opool,
            tc.tile_pool(name="ps", bufs=8, space="PSUM") as pspool,
        ):
            # --- PE warmup: flip HAM to 8/8 during preamble + first DMAs ---
            warm = wpool.tile([P, SW], D, tag="wm", name="warm")
            nc.vector.memset(warm[:], 0)
            wps = pspool.tile([P, SW], mybir.dt.float32, tag="ps", name="ps")
            for _ in range(N_WARM):
                nc.tensor.matmul(
                    wps[:], warm[:, :P], warm[:], start=True, stop=True
                )

            # --- A shard: 4 chunks by k-group; g0/g1 up front, g2/g3 JIT ---
            a_tiles = [None] * 4
            a_starts = [_aoff(8 * g) for g in range(5)]

            def load_a(g):
                ag = apool.tile(
                    [P, a_starts[g + 1] - a_starts[g]], D, tag=f"a{g}", name="ag"
                )
                nc.scalar.dma_start(ag[:], AT[:, a_starts[g] : a_starts[g + 1]])
                a_tiles[g] = ag

            load_a(0)
            # hold the later A chunks out of the congested early window
            # (g1/g2/g3 data is first used at ~13us / ~28us / ~42us)
            with tc.tile_wait_until(0.007):
                load_a(1)
            with tc.tile_wait_until(0.014):
                load_a(2)
            with tc.tile_wait_until(0.020):
                load_a(3)

            def a_sl(k, t):
                g = k // 8
                c0 = _aoff(k) - a_starts[g] + 128 * t
                return a_tiles[g][:, c0 : c0 + 128]

            for vi, v in enumerate(VORDER):
                kmax = _kmax(v)
                nslots = 2 * v + 2
                psums = [
                    pspool.tile([P, SW], mybir.dt.float32, tag="ps", name="ps")
                    for _ in range(nslots)
                ]
                bb = BCHUNKS[v]
                for ci, (kc, kend) in enumerate(zip(bb, bb[1:])):
                    cnt = kend - kc
                    c0 = _BBASE[v] + _boff(kc, v)
                    c1 = _BBASE[v] + _boff(kc + cnt, v)
                    bt = bpool.tile([P, BMAXK * SW], D, tag="b", name="bt")
                    nc.sync.dma_start(bt[:, : c1 - c0], Bm[:, c0:c1])
                    for k in range(kc, kc + cnt):
                        w0 = 128 * _j0(k, v)
                        b0 = _boff(k, v) - _boff(kc, v)
                        bw = 128 * _wtiles(k, v)
                        for t in range(k // 4 + 1):
                            nc.tensor.matmul(
                                psums[t][:, w0:SW],
                                a_sl(k, t),
                                bt[:, b0 : b0 + bw],
                                start=(k == 4 * t),
                                stop=(k == kmax),
                            )
                # drain PSUM -> SBUF (bf16), alternating Vector/Scalar, then
                # store in two batched DMAs on the GpSimd queue
                ot = opool.tile([P, OT_COLS], D, tag="o", name="ot")
                base = _CBASE[(v, 0)]
                for t in range(nslots):
                    w0 = 128 * _j0(4 * t, v)
                    cw = _cwidth(v, t)
                    l0 = _CBASE[(v, t)] - base
                    if t % 2 == 0:
                        nc.vector.tensor_copy(
                            ot[:, l0 : l0 + cw], psums[t][:, w0:SW]
                        )
                    else:
                        nc.scalar.activation(
                            ot[:, l0 : l0 + cw], psums[t][:, w0:SW], Copy
                        )
                half = nslots // 2
                lmid = _CBASE[(v, half)] - base
                lend = _CBASE[(v, nslots - 1)] - base + _cwidth(v, nslots - 1)
                # early vsupers' stores wait out the congested mid-kernel
                # window (B + late-A streaming); later ones go immediately
                cwait = {0: 0.042, 1: 0.050}.get(vi)
                ceng = nc.sync if vi == 2 else nc.gpsimd
                with tc.tile_wait_until(cwait or 0, enable=cwait is not None):
                    ceng.dma_start(Cm[:, base : base + lmid], ot[:, :lmid])
                    ceng.dma_start(
                        Cm[:, base + lmid : base + lend], ot[:, lmid:lend]
                    )
    nc.compile()
    return nc


def _get_nc():
    if "nc" not in _cache:
        _cache["nc"] = _build()
    return _cache["nc"]


def _make_in_maps(A, B):
    import ml_dtypes

    bf16 = np.dtype(ml_dtypes.bfloat16)
    A = np.asarray(A, dtype=np.float32)
    B = np.asarray(B, dtype=np.float32)
    Au = np.triu(A).astype(bf16)
    Bu = np.triu(B).astype(bf16)

    # A packs per row-group r: [p, k-major slots]
    a_packs = []
    for r in range(4):
        ATr = np.zeros((P, A_COLS), dtype=bf16)
        for k in range(KT):
            base = _aoff(k)
            for t in range(k // 4 + 1):
                m = 4 * t + r
                # lhsT[p, ml] = Au[128*m + ml, 128*k + p]
                ATr[:, base + 128 * t : base + 128 * (t + 1)] = Au[
                    128 * m : 128 * m + 128, 128 * k : 128 * k + 128
                ].T
        a_packs.append(ATr)

    # B packs per column parity h
    b_packs = []
    for h in range(2):
        Bh = np.zeros((P, B_COLS), dtype=bf16)
        for v in range(NV):
            for k in range(_kmax(v) + 1):
                base = _BBASE[v] + _boff(k, v)
                for i, j in enumerate(range(_j0(k, v), 4)):
                    n = 8 * v + 2 * j + h
                    Bh[:, base + 128 * i : base + 128 * (i + 1)] = Bu[
                        128 * k : 128 * k + 128, 128 * n : 128 * n + 128
                    ]
        b_packs.append(Bh)

    in_maps = []
    for j in range(NCORES):
        r, h = j % 4, j // 4
        in_maps.append({"AT": a_packs[r], "B": b_packs[h]})
    return in_maps


def kernel(A, B):
    from concourse.bass_utils import run_bass_kernel_spmd

    in_maps = _make_in_maps(A, B)
    nc = _get_nc()
    res = run_bass_kernel_spmd(nc, in_maps, core_ids=list(range(NCORES)))

    C = np.zeros((N, N), dtype=np.float32)
    for jcore in range(NCORES):
        r, h = jcore % 4, jcore // 4
        Cj = np.asarray(res.results[jcore]["C"]).astype(np.float32)
        for v in range(NV):
            for t in range(2 * v + 2):
                m = 4 * t + r
                cb = _CBASE[(v, t)]
                for i, j in enumerate(range(_j0(4 * t, v), 4)):
                    n = 8 * v + 2 * j + h
                    C[128 * m : 128 * m + 128, 128 * n : 128 * n + 128] = Cj[
                        :, cb + 128 * i : cb + 128 * (i + 1)
                    ]
    return C
